# revision 1
# baseline (speedup 1.0000x reference)
"""Trainium2 Bass kernel for nn_CrossAttentionFusion (V=3, B=8192, H=2048, NH=16).

Strategy:
  - Data-parallel: batch B=8192 split across 8 NeuronCores (Bc=1024 each).
  - Feature-major activations on device: every tensor is [H, Bc] so all
    projections are PE matmuls (lhsT = W^T tile [128h x 128g], moving = act
    [128h x 512b]) with no on-device transposes.  Host transposes views and
    weights once (pure layout, no math).
  - fp32r matmuls (TF32-class rounding, 1 cyc/row at N=512 -> ~238ns/MM).
  - Softmax over V-1=2 key views collapses to a sigmoid:
        a0 = sigmoid((qh . (kh0 - kh1)) / sqrt(HD))
        ctx = v2_1 + a0 * (v2_0 - v2_1)
    so the k-side inner projection only needs Wik @ (k[s0] - k[s1]).
  - 27 HxH matmul-equivalents per core, PE-bound.
"""

import math

import numpy as np

V = 3
B = 8192
H = 2048
NH = 16
HD = H // NH
EPS = 1e-5
N_CORES = 8
BC = B // N_CORES          # 1024 batch columns per core
NT = H // 128              # 16 h-tiles
HALF = 512                 # matmul moving free dim
SCALE = 1.0 / math.sqrt(HD)

# others[i] = sources of keys/values for query view i
S0 = [1, 0, 0]
S1 = [2, 2, 1]

_CACHE = {}



def _build_program():
    import concourse.bass as bass
    import concourse.bacc as bacc
    import concourse.tile as tile
    import concourse.mybir as mybir

    f32 = mybir.dt.float32
    f32r = mybir.dt.float32r
    AF = mybir.ActivationFunctionType
    ALU = mybir.AluOpType

    nc = bacc.Bacc("TRN2", target_bir_lowering=False, debug=False,
                   num_devices=N_CORES)

    # ---- External I/O ----
    xT = nc.dram_tensor("xT", [V, H, BC], f32r, kind="ExternalInput").ap()
    wq = nc.dram_tensor("wq", [V, H, H], f32r, kind="ExternalInput").ap()
    wk = nc.dram_tensor("wk", [V, H, H], f32r, kind="ExternalInput").ap()
    wv = nc.dram_tensor("wv", [V, H, H], f32r, kind="ExternalInput").ap()
    wiq = nc.dram_tensor("wiq", [V, H, H], f32r, kind="ExternalInput").ap()
    wik = nc.dram_tensor("wik", [V, H, H], f32r, kind="ExternalInput").ap()
    wiv = nc.dram_tensor("wiv", [V, H, H], f32r, kind="ExternalInput").ap()
    wo = nc.dram_tensor("wo", [V, H, H], f32r, kind="ExternalInput").ap()
    wout = nc.dram_tensor("wout", [V, H, H], f32r, kind="ExternalInput").ap()
    # bias pack: [21,128,16]: bq(0-2) bk(3-5) bv(6-8) biq(9-11) biv(12-14)
    # bo(15-17) bout(18) gamma(19) beta(20); [:, :, gt] is per-partition
    bpk = nc.dram_tensor("bpk", [21, 128, NT], f32, kind="ExternalInput").ap()
    onesc = nc.dram_tensor("onesc", [128, 128], f32r, kind="ExternalInput").ap()
    out = nc.dram_tensor("out", [H, BC], f32, kind="ExternalOutput").ap()

    # ---- DRAM scratch ----
    def scr(name):
        return nc.dram_tensor(name, [V, H, BC], f32r).ap()

    k_s, v_s = scr("k_s"), scr("v_s")
    q2_s, dk2_s = scr("q2_s"), scr("dk2_s")
    dv2_s, v21_s = scr("dv2_s"), scr("v21_s")
    xacc = nc.dram_tensor("xacc", [H, BC], f32r).ap()
    a0_d = nc.dram_tensor("a0_d", [V, NT, BC], f32r).ap()

    with tile.TileContext(nc) as tc:
        ctxs = []

        def pool(name, bufs):
            p = tc.tile_pool(name=name, bufs=bufs)
            ctxs.append(p)
            return p.__enter__()

        xin = pool("xin", 1)        # 16 tags x 4KB (64KB/p)
        res2 = pool("res2", 1)      # 16 tags x 4KB + 2 ln tags (72KB/p)
        wp = pool("wp", 1)          # 16 tags x 1KB (16KB/p)
        stp = pool("stp", 5)        # 1 tag x 5 x 4KB (20KB/p)
        bcp = pool("bcp", 2)        # 1 tag x 2 x 4KB (8KB/p)
        tmp = pool("tmp", 3)        # 1 tag x 3 x 4KB (12KB/p)
        evp = pool("evp", 2)        # 1 tag x 2 x 2KB (4KB/p)
        a0p = pool("a0p", 3)        # 1 tag x 3 x 2KB (6KB/p)
        cst = pool("cst", 1)        # constants (~1.5KB/p)
        psp = tc.tile_pool(name="psp", bufs=1, space="PSUM")
        ctxs.append(psp)
        psp = psp.__enter__()

        # constants
        bias_sb = cst.tile([128, 21, NT], f32)
        nc.sync.dma_start(bias_sb[:], bpk.rearrange("s p f -> p s f"))
        ones_r = cst.tile([128, 1], f32r)
        nc.sync.dma_start(ones_r[:], onesc[:, 0:1])
        ones1_f = cst.tile([1, 128], f32)
        nc.vector.memset(ones1_f[:], 1.0)
        ones1_r = cst.tile([1, 128], f32r)
        nc.sync.dma_start(ones1_r[:], onesc[0:1, :])
        eps_t = cst.tile([1, 1], f32)
        nc.vector.memset(eps_t[:], EPS)

        # residual accumulator starts as views[0] (feature-major); the three
        # Wout partial products are DMA-accumulated into it during P5
        nc.sync.dma_start(xacc[:, :], xT[0])

        # the two resident 16-tile sets; phases ping-pong between them so a
        # build into one set overlaps the projection reading the other
        def rtile(which, t, dt=f32r, name=None):
            pl, tg = (xin, "x") if which == 0 else (res2, "r")
            return pl.tile([128, BC], dt, tag=f"{tg}{t}", name=name or f"{tg}t{t}")

        def load16(src2d, which):
            ts = []
            for t in range(NT):
                tl = rtile(which, t)
                nc.sync.dma_start(tl[:], src2d[t * 128:(t + 1) * 128, :])
                ts.append(tl)
            return ts

        def evict(dst_ap, ps_ap, bidx, gt):
            if bidx is None:
                nc.scalar.activation(dst_ap, ps_ap, AF.Copy)
            else:
                nc.scalar.activation(dst_ap, ps_ap, AF.Identity,
                                     bias=bias_sb[:, bidx, gt:gt + 1])

        def proj(w2d, xt, dst, bidx=None, dst_sb=None, also_dst=None,
                 accum=False):
            """dst[g,b] = sum_h w2d[h,g] x[h,b] (+bias[g]).

            2-g-tile PSUM groups on banks pp0-3 only (pp4-7 stay free for
            the attention/LN small matmuls).  dst_sb: write into SBUF tiles;
            also_dst: additionally DMA dst_sb out to DRAM.
            """
            for gg in range(8):
                wb = []
                for ht in range(NT):
                    w = wp.tile([128, 256], f32r, tag=f"w{ht}", name=f"w{ht}")
                    nc.sync.dma_start(
                        w[:], w2d[ht * 128:(ht + 1) * 128,
                                  gg * 256:(gg + 1) * 256])
                    wb.append(w)
                for hf in range(2):
                    sl = slice(hf * HALF, (hf + 1) * HALF)
                    pts = [psp.tile([128, 512], f32, tag=f"pp{gi + 2 * hf}",
                                    name=f"pt{gi}") for gi in range(2)]
                    for ht in range(NT):
                        for gi in range(2):
                            nc.tensor.matmul(
                                pts[gi][:],
                                wb[ht][:, gi * 128:(gi + 1) * 128],
                                xt[ht][:, sl],
                                start=(ht == 0), stop=(ht == NT - 1))
                    for gi in range(2):
                        gt = gg * 2 + gi
                        if dst_sb is not None:
                            evict(dst_sb[gt][:, sl], pts[gi][:], bidx, gt)
                            if also_dst is not None:
                                nc.sync.dma_start(
                                    also_dst[gt * 128:(gt + 1) * 128, sl],
                                    dst_sb[gt][:, sl])
                        else:
                            et = evp.tile([128, 512], f32r, tag="ev",
                                          name="ev")
                            evict(et[:], pts[gi][:], bidx, gt)
                            if accum:
                                nc.gpsimd.dma_start(
                                    dst[gt * 128:(gt + 1) * 128, sl], et[:],
                                    accum_op=ALU.add)
                            else:
                                nc.sync.dma_start(
                                    dst[gt * 128:(gt + 1) * 128, sl], et[:])

        # ===== P1: per view: k, v (kept + spilled), v21s, q, q2 =====
        for v in range(V):
            xt = load16(xT[v], 0)
            proj(wk[v], xt, k_s[v], bidx=3 + v)
            if v == 0:
                proj(wv[v], xt, v_s[v], bidx=6 + v)
            else:
                vres = [rtile(1, t, name=f"vres{t}") for t in range(NT)]
                proj(wv[v], xt, None, bidx=6 + v, dst_sb=vres,
                     also_dst=v_s[v])
                for i in range(V):
                    if S1[i] == v:
                        proj(wiv[i], vres, v21_s[i], bidx=12 + i)
            qres = [rtile(1, t, name=f"qres{t}") for t in range(NT)]
            proj(wq[v], xt, None, bidx=0 + v, dst_sb=qres)
            proj(wiq[v], qres, q2_s[v], bidx=9 + v)

        # ===== P3: dk2 = Wik @ (k[s0]-k[s1]); sets ping-pong 0,1,0 =====
        for i in range(V):
            which = i % 2
            kd = []
            for t in range(NT):
                k0 = stp.tile([128, BC], f32r, tag="st", name="k0")
                k1 = stp.tile([128, BC], f32r, tag="st", name="k1")
                nc.sync.dma_start(k0[:], k_s[S0[i]][t * 128:(t + 1) * 128, :])
                nc.sync.dma_start(k1[:], k_s[S1[i]][t * 128:(t + 1) * 128, :])
                kt = rtile(which, t, name=f"kd{t}")
                eng = nc.vector if t % 2 == 0 else nc.gpsimd
                eng.tensor_tensor(kt[:], k0[:], k1[:], ALU.subtract)
                kd.append(kt)
            proj(wik[i], kd, dk2_s[i])
            # attention scores: a0 = sigmoid(colsum(q2*dk2)/sqrt(HD)) -> DRAM.
            # Uses only PSUM banks pp4-7, so it fills PE bubbles without
            # contending with the projection pipeline on pp0-3.
            for t in range(NT):
                q2t = stp.tile([128, BC], f32r, tag="st", name="q2t")
                dkt = stp.tile([128, BC], f32r, tag="st", name="dkt")
                nc.sync.dma_start(q2t[:], q2_s[i][t * 128:(t + 1) * 128, :])
                nc.sync.dma_start(dkt[:], dk2_s[i][t * 128:(t + 1) * 128, :])
                pt = tmp.tile([128, BC], f32r, tag="tm", name="pt")
                en2 = nc.vector if t % 2 == 0 else nc.gpsimd
                en2.tensor_tensor(pt[:], q2t[:], dkt[:], ALU.mult)
                for hf in range(2):
                    sl = slice(hf * HALF, (hf + 1) * HALF)
                    cs = psp.tile([128, 512], f32,
                                  tag=f"pp{4 + (2 * t + hf) % 4}", name="cs")
                    nc.tensor.matmul(cs[0:1, :], ones_r[:], pt[:, sl],
                                     start=True, stop=True)
                    a0t = a0p.tile([1, 512], f32r, tag="a0", name="a0t")
                    nc.scalar.activation(a0t[:], cs[0:1, :], AF.Sigmoid,
                                         scale=SCALE)
                    nc.sync.dma_start(a0_d[i, t:t + 1, sl], a0t[:])

        # ===== P4: dv2 = Wiv @ (v[s0]-v[s1]); sets ping-pong 1,0,1 =====
        for i in range(V):
            which = (i + 1) % 2
            vd = []
            for t in range(NT):
                v0 = stp.tile([128, BC], f32r, tag="st", name="v0")
                v1 = stp.tile([128, BC], f32r, tag="st", name="v1")
                nc.sync.dma_start(v0[:], v_s[S0[i]][t * 128:(t + 1) * 128, :])
                nc.sync.dma_start(v1[:], v_s[S1[i]][t * 128:(t + 1) * 128, :])
                vdt = rtile(which, t, name=f"vd{t}")
                eng = nc.vector if t % 2 == 0 else nc.gpsimd
                eng.tensor_tensor(vdt[:], v0[:], v1[:], ALU.subtract)
                vd.append(vdt)
            proj(wiv[i], vd, dv2_s[i])

        # ===== P5: attention (A) + Wo (B) + Wout (C), interleaved =====
        # ctx -> set 0 (xin), att -> set 1 (res2).  A(i) hides under C(i-1);
        # colsums use dedicated PSUM banks pp6/pp7; a0 broadcast via DMA.
        def attn_ctx(i):
            # ctx = v21 + a0*(dv2): no PE/PSUM usage at all -- a0 comes back
            # from DRAM through a partition-broadcast DMA, elementwise work
            # is split halves across DVE and GpSimd.
            ctx_t = []
            h0 = slice(0, HALF)
            h1 = slice(HALF, BC)
            for t in range(NT):
                dvt = stp.tile([128, BC], f32r, tag="st", name="dvt")
                v1t = stp.tile([128, BC], f32r, tag="st", name="v1t")
                nc.sync.dma_start(dvt[:], dv2_s[i][t * 128:(t + 1) * 128, :])
                nc.sync.dma_start(v1t[:], v21_s[i][t * 128:(t + 1) * 128, :])
                bct = bcp.tile([128, BC], f32r, tag="bc", name="bct")
                src = a0_d[i, t]
                a0b = bass.AP(tensor=src.tensor, offset=src.offset,
                              ap=[[0, 128], [1, BC]])
                nc.sync.dma_start(bct[:], a0b)
                t2 = tmp.tile([128, BC], f32r, tag="tm", name="t2")
                ct = rtile(0, t, name=f"ctx{t}")
                nc.vector.tensor_tensor(t2[:, h0], dvt[:, h0], bct[:, h0],
                                        ALU.mult)
                nc.vector.tensor_tensor(ct[:, h0], t2[:, h0], v1t[:, h0],
                                        ALU.add)
                nc.gpsimd.tensor_tensor(t2[:, h1], dvt[:, h1], bct[:, h1],
                                        ALU.mult)
                nc.gpsimd.tensor_tensor(ct[:, h1], t2[:, h1], v1t[:, h1],
                                        ALU.add)
                ctx_t.append(ct)
            return ctx_t

        def proj_B(i, ctx_t):
            att = [rtile(1, t, name=f"att{t}") for t in range(NT)]
            proj(wo[i], ctx_t, None, bidx=15 + i, dst_sb=att)
            return att

        def proj_C(i, att):
            proj(wout[i], att, xacc, bidx=(18 if i == 0 else None),
                 accum=True)

        att_prev = proj_B(0, attn_ctx(0))
        for i in range(1, V):
            proj_C(i - 1, att_prev)
            att_prev = proj_B(i, attn_ctx(i))
        proj_C(V - 1, att_prev)

        # ===== P6: residual + LayerNorm (feature-dim stats via PE) =====
        xln = []
        sx = [psp.tile([128, 512], f32, tag="pp4", name="sx0"),
              psp.tile([128, 512], f32, tag="pp5", name="sx1")]
        sxx = [psp.tile([128, 512], f32, tag="pp6", name="sxx0"),
               psp.tile([128, 512], f32, tag="pp7", name="sxx1")]
        for t in range(NT):
            eng = nc.vector if t % 2 == 0 else nc.gpsimd
            xt = rtile(0, t, name=f"xln{t}")
            nc.sync.dma_start(xt[:], xacc[t * 128:(t + 1) * 128, :])
            sq = tmp.tile([128, BC], f32r, tag="tm", name="sq")
            eng.tensor_tensor(sq[:], xt[:], xt[:], ALU.mult)
            for hf in range(2):
                sl = slice(hf * HALF, (hf + 1) * HALF)
                nc.tensor.matmul(sx[hf][0:1, :], ones_r[:], xt[:, sl],
                                 start=(t == 0), stop=(t == NT - 1))
                nc.tensor.matmul(sxx[hf][0:1, :], ones_r[:], sq[:, sl],
                                 start=(t == 0), stop=(t == NT - 1))
            xln.append(xt)
        mu = res2.tile([1, BC], f32, tag="ln0", name="mu")
        m2 = res2.tile([1, BC], f32, tag="ln1", name="m2")
        for hf in range(2):
            sl = slice(hf * HALF, (hf + 1) * HALF)
            nc.scalar.activation(mu[:, sl], sx[hf][0:1, :], AF.Copy,
                                 scale=1.0 / H)
            nc.scalar.activation(m2[:, sl], sxx[hf][0:1, :], AF.Copy,
                                 scale=1.0 / H)
            msq = a0p.tile([1, 512], f32, tag="a0", name="msq")
            nc.vector.tensor_tensor(msq[:], mu[:, sl], mu[:, sl], ALU.mult)
            nc.vector.tensor_tensor(m2[:, sl], m2[:, sl], msq[:],
                                    ALU.subtract)
        nc.scalar.activation(m2[:], m2[:], AF.Sqrt, bias=eps_t[:])
        nc.vector.reciprocal(m2[:], m2[:])          # rstd
        nc.vector.tensor_tensor(mu[:], mu[:], m2[:], ALU.mult)
        nc.scalar.activation(mu[:], mu[:], AF.Copy, scale=-1.0)  # -mu*rstd
        A_sb = rtile(1, 0, dt=f32, name="Asb")
        B_sb = rtile(1, 1, dt=f32, name="Bsb")
        for hf in range(2):
            sl = slice(hf * HALF, (hf + 1) * HALF)
            pa = psp.tile([128, 512], f32, tag="pp0", name="pa")
            nc.tensor.matmul(pa[:], ones1_f[:], m2[:, sl], start=True,
                             stop=True)
            nc.scalar.activation(A_sb[:, sl], pa[:], AF.Copy)
            pb = psp.tile([128, 512], f32, tag="pp1", name="pb")
            nc.tensor.matmul(pb[:], ones1_f[:], mu[:, sl], start=True,
                             stop=True)
            nc.scalar.activation(B_sb[:, sl], pb[:], AF.Copy)
        for t in range(NT):
            eng = nc.vector if t % 2 == 0 else nc.gpsimd
            n1 = tmp.tile([128, BC], f32, tag="tm", name="n1")
            eng.tensor_tensor(n1[:], xln[t][:].bitcast(f32), A_sb[:],
                              ALU.mult)
            eng.tensor_tensor(n1[:], n1[:], B_sb[:], ALU.add)
            eng.tensor_scalar(
                out=n1[:], in0=n1[:],
                scalar1=bias_sb[:, 19, t:t + 1],
                scalar2=bias_sb[:, 20, t:t + 1],
                op0=ALU.mult, op1=ALU.add)
            nc.sync.dma_start(out[t * 128:(t + 1) * 128, :], n1[:])

        for p in reversed(ctxs):
            p.__exit__(None, None, None)

    nc.compile()
    return nc


def _prep_host(inputs):
    """Transpose/pack host inputs (layout only, no math)."""
    views = np.asarray(inputs["views"], np.float32)

    def t3(a):
        return np.ascontiguousarray(np.asarray(a, np.float32).transpose(0, 2, 1))

    w = {
        "wq": t3(inputs["Wq"]), "wk": t3(inputs["Wk"]), "wv": t3(inputs["Wv"]),
        "wiq": t3(inputs["Wiq"]), "wik": t3(inputs["Wik"]),
        "wiv": t3(inputs["Wiv"]), "wo": t3(inputs["Wo"]),
        "wout": np.ascontiguousarray(
            np.asarray(inputs["Wout"], np.float32).T.reshape(V, H, H)),
    }

    def bcol(vec):
        return np.asarray(vec, np.float32).reshape(NT, 128).T

    bp = np.zeros((21, 128, NT), np.float32)
    for v in range(V):
        bp[0 + v] = bcol(inputs["bq"][v])
        bp[3 + v] = bcol(inputs["bk"][v])
        bp[6 + v] = bcol(inputs["bv"][v])
        bp[9 + v] = bcol(inputs["biq"][v])
        bp[12 + v] = bcol(inputs["biv"][v])
        bp[15 + v] = bcol(inputs["bo"][v])
    bp[18] = bcol(inputs["bout"])
    bp[19] = bcol(inputs["gamma"])
    bp[20] = bcol(inputs["beta"])
    w["bpk"] = bp
    w["onesc"] = np.ones((128, 128), np.float32)

    xts = []
    for c in range(N_CORES):
        sl = views[:, c * BC:(c + 1) * BC, :]
        xts.append(np.ascontiguousarray(sl.transpose(0, 2, 1)))
    return w, xts


def kernel(**inputs):
    from concourse.bass_utils import run_bass_kernel_spmd

    trace = bool(_CACHE.get("trace", False))
    if "nc" not in _CACHE:
        _CACHE["nc"] = _build_program()
    nc = _CACHE["nc"]

    w, xts = _prep_host(inputs)
    in_maps = []
    for c in range(N_CORES):
        m = dict(w)
        m["xT"] = xts[c]
        in_maps.append(m)

    res = run_bass_kernel_spmd(nc, in_maps, core_ids=list(range(N_CORES)),
                               trace=trace)
    _CACHE["last_result"] = res

    outp = np.empty((B, H), np.float32)
    for c in range(N_CORES):
        outp[c * BC:(c + 1) * BC, :] = res.results[c]["out"].T
    return outp



# revision 9
# speedup vs baseline: 1.9673x; 1.9673x over previous
"""Trainium2 Bass kernel for nn_CrossAttentionFusion (V=3, B=8192, H=2048, NH=16).

Strategy (v2 — fused weights):
  - Data-parallel: batch B=8192 split across 8 NeuronCores (1024 each),
    processed in 2 chunks of Bc=512 columns so every intermediate stays
    SBUF-resident (no DRAM spills).
  - Weight fusion on host (constant folding of back-to-back Linears):
        q2  = (Wiq Wq) x_i                          1 pass
        dk2 = (Wik Wk_s0) x_s0 - (Wik Wk_s1) x_s1   2 passes (PSUM-accum,
              second weight negated on host, biases folded)
        v2j = (Wiv Wv_sj) x_sj                      2 passes
        out += (Wout_i Wo_i) ctx                    1 pass
    -> 18 HxH matmul passes/core vs 27 in the unfused version.
  - Softmax over V-1=2 key views == sigmoid:
        a0 = sigmoid((q2 . dk2)/sqrt(HD)) per head (head == 128-row tile)
        ctx = v21 + a0*(v20 - v21)
  - All matmuls in bf16 (same PE rate as fp32r, half the DMA bytes);
    PSUM accumulation fp32; residual/LN path fp32.
  - Residual views[0] (+ fused output bias) is added during the i=0
    output-pass eviction (DVE reads PSUM directly), then i=1,2 are
    DMA-accumulated into DRAM xacc; LayerNorm streams xacc twice.
"""

import math

import numpy as np

V = 3
B = 8192
H = 2048
NH = 16
HD = H // NH
EPS = 1e-5
N_CORES = 8
BPC = B // N_CORES         # 1024 batch columns per core
NCH = 2                    # chunks per core
BC = BPC // NCH            # 512 batch columns per chunk
NT = H // 128              # 16 h-tiles (== heads)
SCALE = 1.0 / math.sqrt(HD)
NP = 6 * V                 # weight passes: per i: q2,k20,k21,v20,v21,uo

# others[i] = sources of keys/values for query view i
S0 = [1, 0, 0]
S1 = [2, 2, 1]

_CACHE = {}


def _build_program():
    import concourse.bass as bass
    import concourse.bacc as bacc
    import concourse.tile as tile
    import concourse.mybir as mybir

    f32 = mybir.dt.float32
    f32r = mybir.dt.float32r
    bf16 = mybir.dt.bfloat16
    AF = mybir.ActivationFunctionType
    ALU = mybir.AluOpType

    nc = bacc.Bacc("TRN2", target_bir_lowering=False, debug=False,
                   num_devices=N_CORES)

    # ---- External I/O ----
    xbf = nc.dram_tensor("xbf", [V, NT, 128, BPC], bf16,
                         kind="ExternalInput").ap()
    # views[0]^T + (Wout_blk0 @ bo0 + bout) broadcast — residual, pre-biased
    x0a = nc.dram_tensor("x0a", [NT, 128, BPC], f32r,
                         kind="ExternalInput").ap()
    # fused lhsT weights, tiled: [pass, gg, hp, ht*256+gc]
    wall = nc.dram_tensor("wall", [NP, 8, 128, NT * 256], bf16,
                          kind="ExternalInput").ap()
    # bias pack: 0-2 bq2, 3-5 bdk, 6-8 bv20, 9-11 bv21, 12-14 bwo(0 unused),
    # 15 gamma, 16 beta
    bpk = nc.dram_tensor("bpk", [17, 128, NT], f32, kind="ExternalInput").ap()
    onesb = nc.dram_tensor("onesb", [128, 2], bf16, kind="ExternalInput").ap()
    onesr = nc.dram_tensor("onesr", [128, 2], f32r, kind="ExternalInput").ap()
    out = nc.dram_tensor("out", [NT, 128, BPC], f32, kind="ExternalOutput").ap()

    # ---- DRAM scratch ----
    a0d = nc.dram_tensor("a0d", [V, NT, BPC], bf16).ap()
    xacc = nc.dram_tensor("xacc", [NT, 128, BPC], f32r).ap()

    with tile.TileContext(nc) as tc:
        ctxs = []

        def pool(name, bufs, space=None):
            kw = dict(name=name, bufs=bufs)
            if space:
                kw["space"] = space
            p = tc.tile_pool(**kw)
            ctxs.append(p)
            return p.__enter__()

        cst = pool("cst", 1)
        xp = pool("xp", 2)        # 48 tags x 1KB x 2          = 96KB
        wp = pool("wp", 2)        # 1 tag  x 8KB x 2           = 16KB
        imq = pool("imq", 1)      # 16 tags x 1KB              = 16KB
        imc = pool("imc", 1)      # 16 tags x 1KB              = 16KB
        dks = pool("dks", 3)      # 1 tag x 1KB x 3            = 3KB
        v2s = pool("v2s", 3)      # 1 tag x 1KB x 3            = 3KB
        bcp = pool("bcp", 4)      # 1 tag x 1KB x 4            = 4KB
        a0p = pool("a0p", 3)      # 1 tag x 1KB x 3            = 3KB
        evp = pool("evp", 3)      # 1 tag x 2KB x 3            = 6KB
        x0p = pool("x0p", 4)      # 1 tag x 2KB x 4            = 8KB
        lnp = pool("lnp", 3)      # 3 tags x 2KB x 3           = 18KB
        stt = pool("stt", 1)      # mu/m2/msq 2KB + A/B 2x2KB  = 10KB
        psp = pool("psp", 1, space="PSUM")

        # constants
        bias_sb = cst.tile([128, 17, NT], f32)
        nc.sync.dma_start(bias_sb[:], bpk.rearrange("s p f -> p s f"))
        ones_b = cst.tile([128, 1], bf16)
        nc.sync.dma_start(ones_b[:], onesb[:, 0:1])
        ones_r = cst.tile([128, 1], f32r)
        nc.sync.dma_start(ones_r[:], onesr[:, 0:1])
        ones1_f = cst.tile([1, 128], f32)
        nc.vector.memset(ones1_f[:], 1.0)
        eps_t = cst.tile([1, 1], f32)
        nc.vector.memset(eps_t[:], EPS)

        def eng(k):
            return nc.vector if k % 2 == 0 else nc.gpsimd

        for c in range(NCH):
            cs = c * BC
            xt = {}   # view -> list of 16 resident x tiles

            def load_x(v):
                ts = []
                for t in range(NT):
                    tl = xp.tile([128, BC], bf16, tag=f"x{v}t{t}",
                                 name=f"x{v}t{t}")
                    nc.sync.dma_start(tl[:], xbf[v, t, :, cs:cs + BC])
                    ts.append(tl)
                xt[v] = ts

            def mm_pass(weights, movings, evict_cb, pre_cb=None):
                """One (or accumulated multi-) HxH projection pass."""
                n = len(weights)
                for gg in range(8):
                    pts = [psp.tile([128, BC], f32, tag=f"pp{2 * (gg % 2) + gi}",
                                    name=f"pt{gi}") for gi in range(2)]
                    if pre_cb is not None:
                        pre_cb(gg)
                    for si in range(n):
                        w = wp.tile([128, NT * 256], bf16, tag="w", name="w")
                        nc.sync.dma_start(w[:], wall[weights[si], gg])
                        mt = movings[si]
                        for ht in range(NT):
                            base = ht * 256
                            for gi in range(2):
                                nc.tensor.matmul(
                                    pts[gi][:],
                                    w[:, base + gi * 128:base + gi * 128 + 128],
                                    mt[ht][:],
                                    start=(si == 0 and ht == 0),
                                    stop=(si == n - 1 and ht == NT - 1))
                    for gi in range(2):
                        evict_cb(gg * 2 + gi, pts[gi])

            for i in range(V):
                p0 = 6 * i
                if i == 0:
                    load_x(i)

                # ---- q2 pass -> imq resident ----
                qt = [None] * NT

                def ev_q2(gt, pt):
                    q = imq.tile([128, BC], bf16, tag=f"q{gt}", name=f"q{gt}")
                    nc.scalar.activation(q[:], pt[:], AF.Identity,
                                         bias=bias_sb[:, 0 + i, gt:gt + 1])
                    qt[gt] = q

                mm_pass([p0 + 0], [xt[i]], ev_q2)

                if i == 0:
                    load_x(S0[i])
                    load_x(S1[i])

                # ---- dk2 pass (2 weights, PSUM-accumulated) + scores ----
                def ev_dk(gt, pt):
                    dk = dks.tile([128, BC], bf16, tag="dk", name="dk")
                    nc.scalar.activation(dk[:], pt[:], AF.Identity,
                                         bias=bias_sb[:, 3 + i, gt:gt + 1])
                    eng(gt).tensor_tensor(dk[:], dk[:], qt[gt][:], ALU.mult)
                    cs_t = psp.tile([128, BC], f32, tag=f"pp{4 + gt % 2}",
                                    name="cs")
                    nc.tensor.matmul(cs_t[0:1, :], ones_b[:], dk[:],
                                     start=True, stop=True)
                    a0t = a0p.tile([1, BC], bf16, tag="a0", name="a0")
                    nc.scalar.activation(a0t[:], cs_t[0:1, :], AF.Sigmoid,
                                         scale=SCALE)
                    nc.sync.dma_start(a0d[i, gt:gt + 1, cs:cs + BC], a0t[:])

                mm_pass([p0 + 1, p0 + 2], [xt[S0[i]], xt[S1[i]]], ev_dk)

                # ---- v20 pass -> imc resident ----
                ct = [None] * NT

                def ev_v20(gt, pt):
                    t_ = imc.tile([128, BC], bf16, tag=f"c{gt}", name=f"c{gt}")
                    nc.scalar.activation(t_[:], pt[:], AF.Identity,
                                         bias=bias_sb[:, 6 + i, gt:gt + 1])
                    ct[gt] = t_

                mm_pass([p0 + 3], [xt[S0[i]]], ev_v20)

                # ---- v21 pass + fused ctx = v21 + a0*(v20-v21) ----
                bct = [None] * NT

                def pre_v21(gg):
                    for gi in range(2):
                        gt = gg * 2 + gi
                        b = bcp.tile([128, BC], bf16, tag="bc", name="bc")
                        src = a0d[i, gt, cs:cs + BC]
                        a0b = bass.AP(tensor=src.tensor, offset=src.offset,
                                      ap=[[0, 128], [1, BC]])
                        nc.sync.dma_start(b[:], a0b)
                        bct[gt] = b

                def ev_v21(gt, pt):
                    v2 = v2s.tile([128, BC], bf16, tag="v2", name="v2")
                    nc.scalar.activation(v2[:], pt[:], AF.Identity,
                                         bias=bias_sb[:, 9 + i, gt:gt + 1])
                    e = eng(gt)
                    e.tensor_tensor(ct[gt][:], ct[gt][:], v2[:], ALU.subtract)
                    e.tensor_tensor(ct[gt][:], ct[gt][:], bct[gt][:], ALU.mult)
                    e.tensor_tensor(ct[gt][:], ct[gt][:], v2[:], ALU.add)

                mm_pass([p0 + 4], [xt[S1[i]]], ev_v21, pre_cb=pre_v21)

                # ---- output pass: xacc (+residual for i=0) ----
                x0t = [None] * NT

                def pre_uo(gg):
                    if i != 0:
                        return
                    for gi in range(2):
                        gt = gg * 2 + gi
                        t_ = x0p.tile([128, BC], f32r, tag="x0", name="x0")
                        nc.sync.dma_start(t_[:], x0a[gt, :, cs:cs + BC])
                        x0t[gt] = t_

                def ev_uo(gt, pt):
                    ev = evp.tile([128, BC], f32r, tag="ev", name="ev")
                    xsl = xacc[gt, :, cs:cs + BC]
                    if i == 0:
                        nc.vector.tensor_tensor(ev[:], pt[:].bitcast(f32r),
                                                x0t[gt][:], ALU.add)
                        nc.sync.dma_start(xsl, ev[:])
                    else:
                        nc.scalar.activation(ev[:], pt[:], AF.Identity,
                                             bias=bias_sb[:, 12 + i,
                                                          gt:gt + 1])
                        nc.gpsimd.dma_start(xsl, ev[:], accum_op=ALU.add)

                mm_pass([p0 + 5], [ct], ev_uo, pre_cb=pre_uo)

            # ===== LayerNorm over features (stream xacc twice) =====
            psx = psp.tile([128, BC], f32, tag="pp6", name="psx")
            psxx = psp.tile([128, BC], f32, tag="pp7", name="psxx")
            for t in range(NT):
                xa = lnp.tile([128, BC], f32r, tag="lna", name="lna")
                nc.sync.dma_start(xa[:], xacc[t, :, cs:cs + BC])
                sq = lnp.tile([128, BC], f32r, tag="lnsq", name="lnsq")
                eng(t).tensor_tensor(sq[:], xa[:], xa[:], ALU.mult)
                nc.tensor.matmul(psx[0:1, :], ones_r[:], xa[:],
                                 start=(t == 0), stop=(t == NT - 1))
                nc.tensor.matmul(psxx[0:1, :], ones_r[:], sq[:],
                                 start=(t == 0), stop=(t == NT - 1))
            mu = stt.tile([1, BC], f32, tag="mu", name="mu")
            m2 = stt.tile([1, BC], f32, tag="m2", name="m2")
            msq = stt.tile([1, BC], f32, tag="msq", name="msq")
            nc.scalar.activation(mu[:], psx[0:1, :], AF.Copy, scale=1.0 / H)
            nc.scalar.activation(m2[:], psxx[0:1, :], AF.Copy, scale=1.0 / H)
            nc.vector.tensor_tensor(msq[:], mu[:], mu[:], ALU.mult)
            nc.vector.tensor_tensor(m2[:], m2[:], msq[:], ALU.subtract)
            nc.scalar.activation(m2[:], m2[:], AF.Sqrt, bias=eps_t[:])
            nc.vector.reciprocal(m2[:], m2[:])             # rstd
            nc.vector.tensor_tensor(mu[:], mu[:], m2[:], ALU.mult)
            nc.scalar.activation(mu[:], mu[:], AF.Copy, scale=-1.0)
            A_sb = stt.tile([128, BC], f32, tag="A", name="A")
            B_sb = stt.tile([128, BC], f32, tag="B", name="B")
            pa = psp.tile([128, BC], f32, tag="pp4", name="pa")
            nc.tensor.matmul(pa[:], ones1_f[:], m2[:], start=True, stop=True)
            nc.scalar.activation(A_sb[:], pa[:], AF.Copy)
            pb = psp.tile([128, BC], f32, tag="pp5", name="pb")
            nc.tensor.matmul(pb[:], ones1_f[:], mu[:], start=True, stop=True)
            nc.scalar.activation(B_sb[:], pb[:], AF.Copy)
            for t in range(NT):
                xa = lnp.tile([128, BC], f32r, tag="lna", name="lna2")
                nc.sync.dma_start(xa[:], xacc[t, :, cs:cs + BC])
                n1 = lnp.tile([128, BC], f32, tag="lnn", name="lnn")
                e = eng(t)
                e.tensor_tensor(n1[:], xa[:].bitcast(f32), A_sb[:], ALU.mult)
                e.tensor_tensor(n1[:], n1[:], B_sb[:], ALU.add)
                e.tensor_scalar(
                    out=n1[:], in0=n1[:],
                    scalar1=bias_sb[:, 15, t:t + 1],
                    scalar2=bias_sb[:, 16, t:t + 1],
                    op0=ALU.mult, op1=ALU.add)
                nc.sync.dma_start(out[t, :, cs:cs + BC], n1[:])

        for p in reversed(ctxs):
            p.__exit__(None, None, None)

    nc.compile()
    return nc


def _prep_host(inputs):
    """Fuse weight pairs (constant folding) + pack layouts on host."""
    import ml_dtypes
    bf = ml_dtypes.bfloat16
    f32 = np.float32

    views = np.asarray(inputs["views"], f32)
    g = {k: np.asarray(inputs[k], f32) for k in
         ("Wq", "bq", "Wk", "bk", "Wv", "bv", "Wiq", "biq", "Wik", "bik",
          "Wiv", "biv", "Wo", "bo", "Wout", "bout", "gamma", "beta")}

    def pack(lhsT):
        # [H_in, H_out] -> [8 gg, 128 hp, (16 ht)*256 gc]
        t = lhsT.reshape(NT, 128, 8, 256).transpose(2, 1, 0, 3)
        return np.ascontiguousarray(t.reshape(8, 128, NT * 256)).astype(bf)

    wlist = []
    bpk = np.zeros((17, 128, NT), f32)

    def bcol(vec):
        return vec.reshape(NT, 128).T

    Wout_blk = [g["Wout"][:, i * H:(i + 1) * H] for i in range(V)]
    for i in range(V):
        wlist.append(pack((g["Wiq"][i] @ g["Wq"][i]).T))
        wlist.append(pack((g["Wik"][i] @ g["Wk"][S0[i]]).T))
        wlist.append(pack(-(g["Wik"][i] @ g["Wk"][S1[i]]).T))
        wlist.append(pack((g["Wiv"][i] @ g["Wv"][S0[i]]).T))
        wlist.append(pack((g["Wiv"][i] @ g["Wv"][S1[i]]).T))
        wlist.append(pack((Wout_blk[i] @ g["Wo"][i]).T))
        bpk[0 + i] = bcol(g["Wiq"][i] @ g["bq"][i] + g["biq"][i])
        bpk[3 + i] = bcol(g["Wik"][i] @ (g["bk"][S0[i]] - g["bk"][S1[i]]))
        bpk[6 + i] = bcol(g["Wiv"][i] @ g["bv"][S0[i]] + g["biv"][i])
        bpk[9 + i] = bcol(g["Wiv"][i] @ g["bv"][S1[i]] + g["biv"][i])
        if i > 0:
            bpk[12 + i] = bcol(Wout_blk[i] @ g["bo"][i])
    bpk[15] = bcol(g["gamma"])
    bpk[16] = bcol(g["beta"])

    shared = {
        "wall": np.ascontiguousarray(np.stack(wlist)),
        "bpk": bpk,
        "onesb": np.ones((128, 2), bf),
        "onesr": np.ones((128, 2), f32),
    }

    # residual (views[0]) pre-biased with the i=0 output-pass bias
    res_bias = (Wout_blk[0] @ g["bo"][0] + g["bout"]).astype(f32)

    percore = []
    for c in range(N_CORES):
        sl = views[:, c * BPC:(c + 1) * BPC, :]          # [V, BPC, H]
        xfm = np.ascontiguousarray(sl.transpose(0, 2, 1))  # [V, H, BPC]
        xbf = xfm.reshape(V, NT, 128, BPC).astype(bf)
        x0 = (xfm[0] + res_bias[:, None]).reshape(NT, 128, BPC)
        percore.append({"xbf": xbf, "x0a": np.ascontiguousarray(x0)})
    return shared, percore


def kernel(**inputs):
    from concourse.bass_utils import run_bass_kernel_spmd

    trace = bool(_CACHE.get("trace", False))
    if "nc" not in _CACHE:
        _CACHE["nc"] = _build_program()
    nc = _CACHE["nc"]

    shared, percore = _prep_host(inputs)
    in_maps = []
    for c in range(N_CORES):
        m = dict(shared)
        m.update(percore[c])
        in_maps.append(m)

    res = run_bass_kernel_spmd(nc, in_maps, core_ids=list(range(N_CORES)),
                               trace=trace)
    _CACHE["last_result"] = res

    outp = np.empty((B, H), np.float32)
    for c in range(N_CORES):
        o = np.asarray(res.results[c]["out"], np.float32).reshape(H, BPC)
        outp[c * BPC:(c + 1) * BPC, :] = o.T
    return outp
